# revision 11
# baseline (speedup 1.0000x reference)
"""Trainium2 Bass kernel for GraphTransitionModel (GNN message passing).

Model (per batch element b, N=256 nodes):
  x[i]   = (obs[b,i], i/N)                              node features, 2-dim
  h1     = relu(A^T x_i + B^T x_j + a*w4 + b0)          messenger layer 1, 64
  h2     = relu(W1^T h1 + b1)                           64
  h3     = relu(W2^T h2 + b2)                           64
  m(i,j) = w3 . h3 + b3                                 scalar
  msg[i] = sum_j m(i,j)
  out    = MLP_updater([x_i, msg[i]])  (3->64->64->64->1)

Strategy: pure data parallel, 4 batch elements per core x 8 cores.

Device layout ("j-loop"): iterate over j; free dim carries all 256 i's.
Two j-rows (j and j+128) are stacked into 128 partitions; the 64x64
layers run as 128x128 block-diagonal bf16 matmuls (1 cycle/col on PE
vs 4 for fp32).  The sum over j is folded into PSUM accumulation of a
tiny third matmul (w3s stationary, start=False across the j-loop), so
no accumulator drains or free-dim reductions are needed.

Elementwise balance per 4-j chunk (free dim 1024):
  DVE: 4x h1 tensor_scalar (bf16 4x mode) + h3 cols [0:HS)
  ACT: h2 relu over [128,1024] + h3 cols [HS:1024)
mm3 reads h3 in 256-col pieces so each matmul sees a single-writer
tile.  The updater MLP runs once at the end over all 4 batches
(free dim 1024, layers 2-4 in bf16).

Sync-wait discipline (single sync-wait slot on matmul): constants via
packed DMAs absorbed by dummy PE matmuls; multi-writer tiles (Pd, Qs)
fenced through single DVE copies before the pair loop reads them.
"""

import os
import sys
import numpy as np

sys.path.insert(0, "/opt/trn_rl_repo")

B, N, MID = 32, 256, 64
NCORES = 8
BPC = B // NCORES  # batches per core = 4
HALF = N // 2  # 128 stacked j-iterations per batch
JCH = 4  # j's per chunk
NCH = HALF // JCH  # 32 chunks per batch
FREE = JCH * N  # 1024 free columns per chunk
HS = 512  # h3 column split: [0:HS) on DVE, [HS:FREE) on ACT

# fp32 wpack column layout
C_W0A = 0
C_W0B = 64
C_UW0 = 128
C_B1S = 192
C_B2S = 193
C_UB0 = 194
C_UB1 = 195
C_UB2 = 196
C_UB3 = 197
C_UW1 = 198
C_UW2 = 262
C_UW3 = 326
C_TOT32 = 327

# bf16 wpack16 column layout
C_W1BD = 0
C_W2BD = 128
C_W3S = 256
C_TOT16 = 258


def _build_bass():
    import concourse.bass as bass
    import concourse.bacc as bacc
    import concourse.tile as tile
    from concourse import mybir

    f32 = mybir.dt.float32
    bf16 = mybir.dt.bfloat16
    AF = mybir.ActivationFunctionType
    ALU = mybir.AluOpType

    nc = bacc.Bacc("TRN2", target_bir_lowering=False, num_devices=NCORES)

    wp_d = nc.declare_dram_parameter("wpack", [128, C_TOT32], f32, isOutput=False)
    wp16_d = nc.declare_dram_parameter("wpack16", [128, C_TOT16], bf16, isOutput=False)
    xT_d = nc.declare_dram_parameter("xT", [BPC, 2, N], f32, isOutput=False)
    ab0_d = nc.declare_dram_parameter("ab0", [BPC, MID, 1], f32, isOutput=False)
    out_d = nc.declare_dram_parameter("out", [BPC, N], f32, isOutput=True)

    with tile.TileContext(nc) as tc:
        with (
            tc.tile_pool(name="consts", bufs=1) as consts,
            tc.tile_pool(name="perb", bufs=2) as perb,
            tc.tile_pool(name="wk1", bufs=6) as wk1,
            tc.tile_pool(name="wk2", bufs=3) as wk2,
            tc.tile_pool(name="wk3a", bufs=3) as wk3a,
            tc.tile_pool(name="wk3b", bufs=3) as wk3b,
            tc.tile_pool(name="wk3s", bufs=3) as wk3s,
            tc.tile_pool(name="wkt", bufs=3) as wkt,
            tc.tile_pool(name="ps_z2", bufs=2, space="PSUM") as ps_z2,
            tc.tile_pool(name="ps_z3", bufs=1, space="PSUM") as ps_z3,
            tc.tile_pool(name="ps_msg", bufs=1, space="PSUM") as ps_msg,
            tc.tile_pool(name="ps_small", bufs=1, space="PSUM") as ps_small,
        ):
            wp = consts.tile([128, C_TOT32], f32, tag="wpack")
            nc.sync.dma_start(out=wp[:], in_=wp_d[:])
            wp16 = consts.tile([128, C_TOT16], bf16, tag="wpack16")
            nc.sync.dma_start(out=wp16[:], in_=wp16_d[:])

            w0a = wp[0:2, C_W0A : C_W0A + MID]
            w0b = wp[0:2, C_W0B : C_W0B + MID]
            uw0 = wp[0:6, C_UW0 : C_UW0 + MID]
            b1s = wp[:, C_B1S : C_B1S + 1]
            b2s = wp[:, C_B2S : C_B2S + 1]
            ub0 = wp[0:MID, C_UB0 : C_UB0 + 1]
            ub1 = wp[0:MID, C_UB1 : C_UB1 + 1]
            ub2 = wp[0:MID, C_UB2 : C_UB2 + 1]
            ub3 = wp[0:1, C_UB3 : C_UB3 + 1]

            w1bd = wp16[:, C_W1BD : C_W1BD + 128]
            w2bd = wp16[:, C_W2BD : C_W2BD + 128]
            w3s = wp16[:, C_W3S : C_W3S + 2]
            uw1 = wp[0:MID, C_UW1 : C_UW1 + MID]
            uw2 = wp[0:MID, C_UW2 : C_UW2 + MID]
            uw3 = wp[0:MID, C_UW3 : C_UW3 + 1]

            # Dummy PE matmuls absorb the two const-DMA waits so later
            # matmuls (single sync-wait slot) only wait on their RAW
            # producer engine.
            psw = ps_small.tile([1, 1], f32, tag="pss")
            nc.tensor.matmul(psw[:], wp[0:1, 0:1], wp[0:1, 0:1], start=True, stop=True)
            psw2 = ps_small.tile([1, 1], f32, tag="pss")
            nc.tensor.matmul(
                psw2[:], wp16[0:1, 0:1], wp16[0:1, 0:1], start=True, stop=True
            )

            # updater input for all batches:
            # rows [obs, coor, msgA_even, msgB_even, msgA_odd, msgB_odd]
            uin = consts.tile([6, BPC * N], f32, tag="uin")

            for b in range(BPC):
                bc = slice(b * N, (b + 1) * N)
                # ---- per-batch setup ----
                nc.sync.dma_start(out=uin[0:2, bc], in_=xT_d[b])
                ab0s = perb.tile([MID, 1], f32, tag="ab0s")
                nc.sync.dma_start(out=ab0s[:], in_=ab0_d[b])

                psP = ps_small.tile([MID, N], f32, tag="pss")
                nc.tensor.matmul(psP[:], w0a, uin[0:2, bc], start=True, stop=True)
                Pd = perb.tile([128, N], bf16, tag="Pd")
                nc.scalar.copy(Pd[0:MID, :], psP[:])
                nc.sync.dma_start(out=Pd[MID:128, :], in_=Pd[0:MID, :])

                psQ = ps_small.tile([MID, N], f32, tag="pss")
                nc.tensor.matmul(psQ[:], w0b, uin[0:2, bc], start=True, stop=True)
                qtmp = perb.tile([MID, N], f32, tag="qtmp")
                nc.scalar.activation(qtmp[:], psQ[:], AF.Identity, bias=ab0s)
                Qs = perb.tile([128, HALF], f32, tag="Qs")
                nc.sync.dma_start(out=Qs[0:MID, :], in_=qtmp[:, 0:HALF])
                nc.sync.dma_start(out=Qs[MID:128, :], in_=qtmp[:, HALF:N])

                # DVE fences: single-writer sources for the pair loop
                Pd2 = perb.tile([128, N], bf16, tag="Pd2")
                nc.vector.tensor_copy(Pd2[:], Pd[:])
                Qs2 = perb.tile([128, HALF], f32, tag="Qs2")
                nc.vector.tensor_copy(Qs2[:], Qs[:])

                # msg accumulator: partitions = j-halves, col blocks = even/odd j
                msg_ps = ps_msg.tile([2, 2 * N], f32, tag="msg")

                # ---- j-loop, 2-stage software pipeline ----
                # body c: DVE: h3a(c-2), h1 TSPs(c);  ACT: h3b(c-2), h2(c-1);
                #         PE: mm1(c-1), mm3(c-2), mm2(c-1)
                # Loop-carried chains never span a full body, so the period
                # is set by the busiest engine, not the dependency loop.
                h1q, z2q, z3q, h3q = {}, {}, {}, {}
                for c in range(NCH + 2):
                    if c >= 2:
                        z3p = z3q.pop(c - 2)
                        h3a = wk3a.tile([128, HS], bf16, tag="h3a")
                        nc.vector.tensor_scalar(
                            h3a[:], z3p[:, 0:HS], b2s, 0.0, ALU.add, ALU.max
                        )
                    if c < NCH:
                        jb = c * JCH
                        h1a = wk1.tile([128, 2 * N], bf16, tag="h1a")
                        nc.vector.tensor_scalar(
                            h1a[:, 0:N], Pd2[:], Qs2[:, jb : jb + 1],
                            0.0, ALU.add, ALU.max,
                        )
                        nc.vector.tensor_scalar(
                            h1a[:, N : 2 * N], Pd2[:], Qs2[:, jb + 1 : jb + 2],
                            0.0, ALU.add, ALU.max,
                        )
                        h1b = wk1.tile([128, 2 * N], bf16, tag="h1b")
                        nc.vector.tensor_scalar(
                            h1b[:, 0:N], Pd2[:], Qs2[:, jb + 2 : jb + 3],
                            0.0, ALU.add, ALU.max,
                        )
                        nc.vector.tensor_scalar(
                            h1b[:, N : 2 * N], Pd2[:], Qs2[:, jb + 3 : jb + 4],
                            0.0, ALU.add, ALU.max,
                        )
                        h1q[c] = (h1a, h1b)
                    if c >= 2:
                        h3b = wk3b.tile([128, FREE - HS], bf16, tag="h3b")
                        nc.scalar.activation(
                            h3b[:], z3p[:, HS:FREE], AF.Relu, bias=b2s
                        )
                        # pre-sum the two h3 halves on GPSIMD so mm3 streams
                        # 512 cols instead of 1024 (PE is LDW/instr-bound)
                        h3s = wk3s.tile([128, HS], bf16, tag="h3s")
                        nc.gpsimd.tensor_tensor(h3s[:], h3a[:], h3b[:], ALU.add)
                        h3q[c - 2] = h3s
                    if 1 <= c <= NCH:
                        p1a, p1b = h1q.pop(c - 1)
                        z2 = ps_z2.tile([128, FREE], f32, tag="z2")
                        nc.tensor.matmul(
                            z2[:, 0 : 2 * N], w1bd, p1a[:], start=True, stop=True
                        )
                        nc.tensor.matmul(
                            z2[:, 2 * N : FREE], w1bd, p1b[:], start=True, stop=True
                        )
                        z2q[c - 1] = z2
                    if c >= 2:
                        p3s = h3q.pop(c - 2)
                        nc.tensor.matmul(
                            msg_ps[:], w3s, p3s[:],
                            start=(c == 2), stop=(c == NCH + 1),
                            skip_group_check=True,
                        )
                    if 1 <= c <= NCH:
                        z2c = z2q.pop(c - 1)
                        h2 = wk2.tile([128, FREE], bf16, tag="h2")
                        nc.scalar.activation(h2[:], z2c[:], AF.Relu, bias=b1s)
                        z3 = ps_z3.tile([128, FREE], f32, tag="z3")
                        nc.tensor.matmul(
                            z3[:, 0 : 2 * N], w2bd, h2[:, 0 : 2 * N],
                            start=True, stop=True,
                        )
                        nc.tensor.matmul(
                            z3[:, 2 * N : FREE], w2bd, h2[:, 2 * N : FREE],
                            start=True, stop=True,
                        )
                        z3q[c - 1] = z3

                # msg PSUM -> SBUF -> uin rows 2:6 (DMA crosses partitions)
                msg_sb = perb.tile([2, 2 * N], f32, tag="msg_sb")
                nc.scalar.copy(msg_sb[:], msg_ps[:])
                nc.sync.dma_start(out=uin[2:4, bc], in_=msg_sb[:, 0:N])
                nc.sync.dma_start(out=uin[4:6, bc], in_=msg_sb[:, N : 2 * N])

            # ---- updater MLP, all batches at once (free = BPC*N) ----
            FB = BPC * N
            psu1 = ps_z2.tile([MID, FB], f32, tag="z2")
            nc.tensor.matmul(
                psu1[:, 0 : FB // 2], uw0, uin[:, 0 : FB // 2], start=True, stop=True
            )
            nc.tensor.matmul(
                psu1[:, FB // 2 : FB], uw0, uin[:, FB // 2 : FB], start=True, stop=True
            )
            t1 = wkt.tile([MID, FB], f32, tag="t1")
            nc.scalar.activation(t1[:], psu1[:], AF.Relu, bias=ub0)
            psu2 = ps_z3.tile([MID, FB], f32, tag="z3")
            nc.tensor.matmul(
                psu2[:, 0 : FB // 2], uw1, t1[:, 0 : FB // 2], start=True, stop=True
            )
            nc.tensor.matmul(
                psu2[:, FB // 2 : FB], uw1, t1[:, FB // 2 : FB], start=True, stop=True
            )
            t2 = wkt.tile([MID, FB], f32, tag="t2")
            nc.scalar.activation(t2[:], psu2[:], AF.Relu, bias=ub1)
            psu3 = ps_z2.tile([MID, FB], f32, tag="z2")
            nc.tensor.matmul(
                psu3[:, 0 : FB // 2], uw2, t2[:, 0 : FB // 2], start=True, stop=True
            )
            nc.tensor.matmul(
                psu3[:, FB // 2 : FB], uw2, t2[:, FB // 2 : FB], start=True, stop=True
            )
            t3 = wkt.tile([MID, FB], f32, tag="t3")
            nc.scalar.activation(t3[:], psu3[:], AF.Relu, bias=ub2)
            pso = ps_z3.tile([1, FB], f32, tag="z3")
            nc.tensor.matmul(
                pso[:, 0 : FB // 2], uw3, t3[:, 0 : FB // 2], start=True, stop=True
            )
            nc.tensor.matmul(
                pso[:, FB // 2 : FB], uw3, t3[:, FB // 2 : FB], start=True, stop=True
            )
            orow = consts.tile([1, FB], f32, tag="orow")
            nc.scalar.activation(orow[:], pso[:], AF.Identity, bias=ub3)
            for b in range(BPC):
                nc.sync.dma_start(
                    out=out_d[b], in_=orow[0:1, b * N : (b + 1) * N]
                )

    nc.compile()
    return nc


def _host_inputs(inputs):
    import ml_dtypes

    g = lambda k: np.asarray(inputs[k], np.float32)
    obs, action = g("obs"), g("action")
    m_w0, m_b0, m_w1, m_b1 = g("m_w0"), g("m_b0"), g("m_w1"), g("m_b1")
    m_w2, m_b2, m_w3, m_b3 = g("m_w2"), g("m_b2"), g("m_w3"), g("m_b3")
    u_w0, u_b0, u_w1, u_b1 = g("u_w0"), g("u_b0"), g("u_w1"), g("u_b1")
    u_w2, u_b2, u_w3, u_b3 = g("u_w2"), g("u_b2"), g("u_w3"), g("u_b3")

    coor = np.arange(N, dtype=np.float32) / N
    xT = np.stack([obs, np.broadcast_to(coor, obs.shape)], axis=1)  # [B, 2, N]
    # per-batch action/bias for Q: a*w4 + b0
    ab0 = (action[:, None] * m_w0[4] + m_b0).astype(np.float32)[..., None]

    wpack = np.zeros((128, C_TOT32), np.float32)
    wpack[0:2, C_W0A : C_W0A + MID] = m_w0[0:2]
    wpack[0:2, C_W0B : C_W0B + MID] = m_w0[2:4]
    wpack[0:2, C_UW0 : C_UW0 + MID] = u_w0[0:2]
    for _r in (2, 3, 4, 5):
        wpack[_r, C_UW0 : C_UW0 + MID] = u_w0[2]
    wpack[:MID, C_B1S] = m_b1
    wpack[MID:, C_B1S] = m_b1
    wpack[:MID, C_B2S] = m_b2
    wpack[MID:, C_B2S] = m_b2
    # msg rows exclude the +N*b3 term; fold it into the updater bias
    wpack[:MID, C_UB0] = u_b0 + N * float(m_b3[0]) * u_w0[2]
    wpack[:MID, C_UB1] = u_b1
    wpack[:MID, C_UB2] = u_b2
    wpack[0, C_UB3] = float(u_b3[0])
    wpack[:MID, C_UW1 : C_UW1 + MID] = u_w1
    wpack[:MID, C_UW2 : C_UW2 + MID] = u_w2
    wpack[:MID, C_UW3] = u_w3[:, 0]

    wpack16 = np.zeros((128, C_TOT16), np.float32)
    wpack16[:MID, C_W1BD : C_W1BD + MID] = m_w1
    wpack16[MID:, C_W1BD + MID : C_W1BD + 128] = m_w1
    wpack16[:MID, C_W2BD : C_W2BD + MID] = m_w2
    wpack16[MID:, C_W2BD + MID : C_W2BD + 128] = m_w2
    wpack16[:MID, C_W3S] = m_w3[:, 0]
    wpack16[MID:, C_W3S + 1] = m_w3[:, 0]
    wpack16 = wpack16.astype(ml_dtypes.bfloat16)

    in_maps = []
    for c in range(NCORES):
        sl = slice(c * BPC, (c + 1) * BPC)
        in_maps.append(
            dict(
                wpack=wpack,
                wpack16=wpack16,
                xT=np.ascontiguousarray(xT[sl]),
                ab0=np.ascontiguousarray(ab0[sl]),
            )
        )
    return in_maps


def kernel(**inputs) -> np.ndarray:
    in_maps = _host_inputs(inputs)

    from concourse.bass_utils import run_bass_kernel_spmd

    nc = _build_bass()
    res = run_bass_kernel_spmd(
        nc, in_maps, core_ids=list(range(NCORES)),
        trace=bool(int(os.environ.get("KERNEL_TRACE", "0"))),
    )
    out = np.concatenate([r["out"] for r in res.results], axis=0)  # [B, N]
    if res.exec_time_ns is not None:
        print(f"HW exec time: {res.exec_time_ns} ns")
        print(f"mean exec time: {res.mean_exec_time_ns} ns")
    return out.astype(np.float32)


if __name__ == "__main__":
    nc = _build_bass()
    print("bass build OK")


# revision 19
# speedup vs baseline: 1.6315x; 1.6315x over previous
"""Trainium2 Bass kernel for GraphTransitionModel (GNN message passing).

Model (per batch element b, N=256 nodes):
  x[i]   = (obs[b,i], i/N)                              node features, 2-dim
  h1     = relu(A^T x_i + B^T x_j + a*w4 + b0)          messenger layer 1, 64
  h2     = relu(W1^T h1 + b1)                           64
  h3     = relu(W2^T h2 + b2)                           64
  m(i,j) = w3 . h3 + b3                                 scalar
  msg[i] = sum_j m(i,j)
  out    = MLP_updater([x_i, msg[i]])  (3->64->64->64->1)

Strategy: pure data parallel, 4 batch elements per core x 8 cores.

Device layout ("j-loop"): iterate over j; free dim carries all 256 i's.
Two j-rows (j and j+128) are stacked into 128 partitions; the 64x64
layers run as 128x128 block-diagonal bf16 matmuls (1 cycle/col on PE
vs 4 for fp32).  The sum over j is folded into PSUM accumulation of a
tiny third matmul (w3s stationary, start=False across the j-loop), so
no accumulator drains or free-dim reductions are needed.

Elementwise balance per 4-j chunk (free dim 1024):
  DVE: 4x h1 tensor_scalar (bf16 4x mode) + h3 cols [0:HS)
  ACT: h2 relu over [128,1024] + h3 cols [HS:1024)
mm3 reads h3 in 256-col pieces so each matmul sees a single-writer
tile.  The updater MLP runs once at the end over all 4 batches
(free dim 1024, layers 2-4 in bf16).

Sync-wait discipline (single sync-wait slot on matmul): constants via
packed DMAs absorbed by dummy PE matmuls; multi-writer tiles (Pd, Qs)
fenced through single DVE copies before the pair loop reads them.
"""

import os
import sys
import numpy as np

sys.path.insert(0, "/opt/trn_rl_repo")

B, N, MID = 32, 256, 64
NCORES = 8
BPC = B // NCORES  # batches per core = 4
HALF = N // 2  # 128 stacked j-iterations per batch
JCH = 4  # j's per chunk
NCH = HALF // JCH  # 32 chunks per batch
FREE = JCH * N  # 1024 free columns per chunk
HS = 512  # h3 column split: [0:HS) on DVE, [HS:FREE) on ACT

# fp32 wpack column layout
C_W0A = 0
C_W0B = 64
C_UW0 = 128
C_B1S = 192
C_B2S = 193
C_UB0 = 194
C_UB1 = 195
C_UB2 = 196
C_UB3 = 197
C_UW1 = 198
C_UW2 = 262
C_UW3 = 326
C_TOT32 = 327

# bf16 wpack16 column layout
C_W1BD = 0
C_W2BD = 128
C_W3S = 256
C_TOT16 = 258


def _build_bass():
    import concourse.bass as bass
    import concourse.bacc as bacc
    import concourse.tile as tile
    from concourse import mybir

    f32 = mybir.dt.float32
    bf16 = mybir.dt.bfloat16
    AF = mybir.ActivationFunctionType
    ALU = mybir.AluOpType

    nc = bacc.Bacc("TRN2", target_bir_lowering=False, num_devices=NCORES)

    wp_d = nc.declare_dram_parameter("wpack", [128, C_TOT32], f32, isOutput=False)
    wp16_d = nc.declare_dram_parameter("wpack16", [128, C_TOT16], bf16, isOutput=False)
    xT_d = nc.declare_dram_parameter("xT", [BPC, 2, N], f32, isOutput=False)
    ab0_d = nc.declare_dram_parameter("ab0", [BPC, MID, 1], f32, isOutput=False)
    out_d = nc.declare_dram_parameter("out", [BPC, N], f32, isOutput=True)

    with tile.TileContext(nc) as tc:
        with (
            tc.tile_pool(name="consts", bufs=1) as consts,
            tc.tile_pool(name="perb", bufs=2) as perb,
            tc.tile_pool(name="wk1", bufs=6) as wk1,
            tc.tile_pool(name="wk2", bufs=3) as wk2,
            tc.tile_pool(name="wk3a", bufs=3) as wk3a,
            tc.tile_pool(name="wk3b", bufs=3) as wk3b,
            tc.tile_pool(name="wkt", bufs=3) as wkt,
            tc.tile_pool(name="ps_z", bufs=3, space="PSUM") as ps_z,
            tc.tile_pool(name="ps_msg", bufs=1, space="PSUM") as ps_msg,
            tc.tile_pool(name="ps_small", bufs=1, space="PSUM") as ps_small,
        ):
            wp = consts.tile([128, C_TOT32], f32, tag="wpack")
            nc.sync.dma_start(out=wp[:], in_=wp_d[:])
            wp16 = consts.tile([128, C_TOT16], bf16, tag="wpack16")
            nc.sync.dma_start(out=wp16[:], in_=wp16_d[:])

            w0a = wp[0:2, C_W0A : C_W0A + MID]
            w0b = wp[0:2, C_W0B : C_W0B + MID]
            uw0 = wp[0:6, C_UW0 : C_UW0 + MID]
            b1s = wp[:, C_B1S : C_B1S + 1]
            b2s = wp[:, C_B2S : C_B2S + 1]
            ub0 = wp[0:MID, C_UB0 : C_UB0 + 1]
            ub1 = wp[0:MID, C_UB1 : C_UB1 + 1]
            ub2 = wp[0:MID, C_UB2 : C_UB2 + 1]
            ub3 = wp[0:1, C_UB3 : C_UB3 + 1]

            w1bd = wp16[:, C_W1BD : C_W1BD + 128]
            w2bd = wp16[:, C_W2BD : C_W2BD + 128]
            w3s = wp16[:, C_W3S : C_W3S + 2]
            uw1 = wp[0:MID, C_UW1 : C_UW1 + MID]
            uw2 = wp[0:MID, C_UW2 : C_UW2 + MID]
            uw3 = wp[0:MID, C_UW3 : C_UW3 + 1]

            # Dummy PE matmuls absorb the two const-DMA waits so later
            # matmuls (single sync-wait slot) only wait on their RAW
            # producer engine.
            psw = ps_small.tile([1, 1], f32, tag="pss")
            nc.tensor.matmul(psw[:], wp[0:1, 0:1], wp[0:1, 0:1], start=True, stop=True)
            psw2 = ps_small.tile([1, 1], f32, tag="pss")
            nc.tensor.matmul(
                psw2[:], wp16[0:1, 0:1], wp16[0:1, 0:1], start=True, stop=True
            )

            # updater input for all batches:
            # rows [obs, coor, msgA_even, msgB_even, msgA_odd, msgB_odd]
            uin = consts.tile([6, BPC * N], f32, tag="uin")

            def emit_setup(b):
                bcs = slice(b * N, (b + 1) * N)
                nc.sync.dma_start(out=uin[0:2, bcs], in_=xT_d[b])
                ab0s = perb.tile([MID, 1], f32, tag="ab0s")
                nc.sync.dma_start(out=ab0s[:], in_=ab0_d[b])

                psP = ps_small.tile([MID, N], f32, tag="pss")
                nc.tensor.matmul(psP[:], w0a, uin[0:2, bcs], start=True, stop=True)
                Pd = perb.tile([128, N], bf16, tag="Pd")
                nc.scalar.copy(Pd[0:MID, :], psP[:])
                nc.sync.dma_start(out=Pd[MID:128, :], in_=Pd[0:MID, :])

                psQ = ps_small.tile([MID, N], f32, tag="pss")
                nc.tensor.matmul(psQ[:], w0b, uin[0:2, bcs], start=True, stop=True)
                qtmp = perb.tile([MID, N], f32, tag="qtmp")
                nc.scalar.activation(qtmp[:], psQ[:], AF.Identity, bias=ab0s)
                Qs = perb.tile([128, HALF], f32, tag="Qs")
                nc.sync.dma_start(out=Qs[0:MID, :], in_=qtmp[:, 0:HALF])
                nc.sync.dma_start(out=Qs[MID:128, :], in_=qtmp[:, HALF:N])

                # DVE fences: single-writer sources for the pair loop
                Pd2 = perb.tile([128, N], bf16, tag="Pd2")
                nc.vector.tensor_copy(Pd2[:], Pd[:])
                Qs2 = perb.tile([128, HALF], f32, tag="Qs2")
                nc.vector.tensor_copy(Qs2[:], Qs[:])

                # msg accumulator: partitions = j-halves, cols = even/odd j
                msg_ps = ps_msg.tile([2, 2 * N], f32, tag="msg")
                return (Pd2, Qs2, msg_ps)

            # ---- flat 4-stage pipeline over all BPC*NCH chunks ----
            # chunk x: quad(x) @ body x [DVE] -> mm1(x) @ x+1 [PE] ->
            # h2(x) @ x+1 [ACT] -> mm2(x) @ x+2 [PE] -> h3(x) @ x+3
            # [DVE+ACT] -> mm3(x) @ x+3 [PE].  Every consumer runs a full
            # body after its producer, so the period is set by the busiest
            # engine, not by cross-engine dependency round trips.
            T = BPC * NCH
            ctx = {0: emit_setup(0)}
            h1q, z2q, h2q, z3q, h3q = {}, {}, {}, {}, {}
            for t in range(T + 4):
                # stage 4a: h3(t-3) relu split DVE/ACT
                if 3 <= t < T + 3:
                    x = t - 3
                    z3p = z3q.pop(x)
                    h3a = wk3a.tile([128, HS], bf16, tag="h3a")
                    nc.vector.tensor_scalar(
                        h3a[:], z3p[:, 0:HS], b2s, 0.0, ALU.add, ALU.max
                    )
                    h3b = wk3b.tile([128, FREE - HS], bf16, tag="h3b")
                    nc.scalar.activation(
                        h3b[:], z3p[:, HS:FREE], AF.Relu, bias=b2s
                    )
                    h3q[x] = (h3a, h3b)
                # stage 1: h1 quad(t) on DVE
                if t < T:
                    bq, cq = divmod(t, NCH)
                    Pd2, Qs2, _ = ctx[bq]
                    jb = cq * JCH
                    h1a = wk1.tile([128, 2 * N], bf16, tag="h1a")
                    nc.vector.tensor_scalar(
                        h1a[:, 0:N], Pd2[:], Qs2[:, jb : jb + 1],
                        0.0, ALU.add, ALU.max,
                    )
                    nc.vector.tensor_scalar(
                        h1a[:, N : 2 * N], Pd2[:], Qs2[:, jb + 1 : jb + 2],
                        0.0, ALU.add, ALU.max,
                    )
                    h1b = wk1.tile([128, 2 * N], bf16, tag="h1b")
                    nc.vector.tensor_scalar(
                        h1b[:, 0:N], Pd2[:], Qs2[:, jb + 2 : jb + 3],
                        0.0, ALU.add, ALU.max,
                    )
                    nc.vector.tensor_scalar(
                        h1b[:, N : 2 * N], Pd2[:], Qs2[:, jb + 3 : jb + 4],
                        0.0, ALU.add, ALU.max,
                    )
                    h1q[t] = (h1a, h1b)
                # stage 2a: mm1(t-1) on PE (z2 single-buffered: its reader
                # h2(t-1) finished a body before mm1(t) rewrites it)
                if 1 <= t <= T:
                    x = t - 1
                    p1a, p1b = h1q.pop(x)
                    z2 = ps_z.tile([128, FREE], f32, tag="z")
                    nc.tensor.matmul(
                        z2[:, 0 : 2 * N], w1bd, p1a[:], start=True, stop=True
                    )
                    nc.tensor.matmul(
                        z2[:, 2 * N : FREE], w1bd, p1b[:], start=True, stop=True
                    )
                    z2q[x] = z2
                # stage 3: mm2(t-2) on PE
                if 2 <= t <= T + 1:
                    x = t - 2
                    h2c = h2q.pop(x)
                    z3 = ps_z.tile([128, FREE], f32, tag="z")
                    nc.tensor.matmul(
                        z3[:, 0 : 2 * N], w2bd, h2c[:, 0 : 2 * N],
                        start=True, stop=True,
                    )
                    nc.tensor.matmul(
                        z3[:, 2 * N : FREE], w2bd, h2c[:, 2 * N : FREE],
                        start=True, stop=True,
                    )
                    z3q[x] = z3
                # stage 4b: mm3(t-3) accumulate + batch drain
                if 3 <= t < T + 3:
                    x = t - 3
                    bx, cx = divmod(x, NCH)
                    p3a, p3b = h3q.pop(x)
                    msg_x = ctx[bx][2]
                    nc.tensor.matmul(
                        msg_x[:], w3s, p3a[:],
                        start=(cx == 0), stop=False, skip_group_check=True,
                    )
                    nc.tensor.matmul(
                        msg_x[:], w3s, p3b[:],
                        start=False, stop=(cx == NCH - 1),
                        skip_group_check=True,
                    )
                    if cx == NCH - 1:
                        # msg PSUM -> SBUF -> uin rows 2:6 (DMA crosses
                        # partitions)
                        bcx = slice(bx * N, (bx + 1) * N)
                        msg_sb = perb.tile([2, 2 * N], f32, tag="msg_sb")
                        nc.scalar.copy(msg_sb[:], msg_x[:])
                        nc.sync.dma_start(out=uin[2:4, bcx], in_=msg_sb[:, 0:N])
                        nc.sync.dma_start(
                            out=uin[4:6, bcx], in_=msg_sb[:, N : 2 * N]
                        )
                        del ctx[bx]
                # stage 2b: h2(t-1) on ACT
                if 1 <= t <= T:
                    x = t - 1
                    z2c = z2q.pop(x)
                    h2 = wk2.tile([128, FREE], bf16, tag="h2")
                    nc.scalar.activation(h2[:], z2c[:], AF.Relu, bias=b1s)
                    h2q[x] = h2
                # prefetch next batch's setup two bodies early
                bn, rn = divmod(t + 2, NCH)
                if rn == 0 and 1 <= bn < BPC:
                    ctx[bn] = emit_setup(bn)

            # ---- updater MLP, all batches at once (free = BPC*N) ----
            # fp32 throughout (bf16 activations here cost 2.8e-2 rel err);
            # two column-halves pipeline the PE matmuls against ACT relus.
            FB = BPC * N
            H = FB // 2
            halves = (slice(0, H), slice(H, FB))
            psu1 = ps_z.tile([MID, FB], f32, tag="z")
            for s in halves:
                nc.tensor.matmul(psu1[:, s], uw0, uin[:, s], start=True, stop=True)
            t1 = wkt.tile([MID, FB], f32, tag="t1")
            psu2 = ps_z.tile([MID, FB], f32, tag="z")
            t2 = wkt.tile([MID, FB], f32, tag="t2")
            psu3 = ps_z.tile([MID, FB], f32, tag="z")
            t3 = wkt.tile([MID, FB], f32, tag="t3")
            pso = ps_z.tile([1, FB], f32, tag="z")
            orow = consts.tile([1, FB], f32, tag="orow")
            for s in halves:
                nc.scalar.activation(t1[:, s], psu1[:, s], AF.Relu, bias=ub0)
                nc.tensor.matmul(psu2[:, s], uw1, t1[:, s], start=True, stop=True)
            for s in halves:
                nc.scalar.activation(t2[:, s], psu2[:, s], AF.Relu, bias=ub1)
                nc.tensor.matmul(psu3[:, s], uw2, t2[:, s], start=True, stop=True)
            for s in halves:
                nc.scalar.activation(t3[:, s], psu3[:, s], AF.Relu, bias=ub2)
                nc.tensor.matmul(pso[:, s], uw3, t3[:, s], start=True, stop=True)
            for s in halves:
                nc.scalar.activation(orow[:, s], pso[:, s], AF.Identity, bias=ub3)
            nc.sync.dma_start(out=out_d[:, :], in_=orow[:])

    nc.compile()
    return nc


def _host_inputs(inputs):
    import ml_dtypes

    g = lambda k: np.asarray(inputs[k], np.float32)
    obs, action = g("obs"), g("action")
    m_w0, m_b0, m_w1, m_b1 = g("m_w0"), g("m_b0"), g("m_w1"), g("m_b1")
    m_w2, m_b2, m_w3, m_b3 = g("m_w2"), g("m_b2"), g("m_w3"), g("m_b3")
    u_w0, u_b0, u_w1, u_b1 = g("u_w0"), g("u_b0"), g("u_w1"), g("u_b1")
    u_w2, u_b2, u_w3, u_b3 = g("u_w2"), g("u_b2"), g("u_w3"), g("u_b3")

    coor = np.arange(N, dtype=np.float32) / N
    xT = np.stack([obs, np.broadcast_to(coor, obs.shape)], axis=1)  # [B, 2, N]
    # per-batch action/bias for Q: a*w4 + b0
    ab0 = (action[:, None] * m_w0[4] + m_b0).astype(np.float32)[..., None]

    wpack = np.zeros((128, C_TOT32), np.float32)
    wpack[0:2, C_W0A : C_W0A + MID] = m_w0[0:2]
    wpack[0:2, C_W0B : C_W0B + MID] = m_w0[2:4]
    wpack[0:2, C_UW0 : C_UW0 + MID] = u_w0[0:2]
    for _r in (2, 3, 4, 5):
        wpack[_r, C_UW0 : C_UW0 + MID] = u_w0[2]
    wpack[:MID, C_B1S] = m_b1
    wpack[MID:, C_B1S] = m_b1
    wpack[:MID, C_B2S] = m_b2
    wpack[MID:, C_B2S] = m_b2
    # msg rows exclude the +N*b3 term; fold it into the updater bias
    wpack[:MID, C_UB0] = u_b0 + N * float(m_b3[0]) * u_w0[2]
    wpack[:MID, C_UB1] = u_b1
    wpack[:MID, C_UB2] = u_b2
    wpack[0, C_UB3] = float(u_b3[0])
    wpack[:MID, C_UW1 : C_UW1 + MID] = u_w1
    wpack[:MID, C_UW2 : C_UW2 + MID] = u_w2
    wpack[:MID, C_UW3] = u_w3[:, 0]

    wpack16 = np.zeros((128, C_TOT16), np.float32)
    wpack16[:MID, C_W1BD : C_W1BD + MID] = m_w1
    wpack16[MID:, C_W1BD + MID : C_W1BD + 128] = m_w1
    wpack16[:MID, C_W2BD : C_W2BD + MID] = m_w2
    wpack16[MID:, C_W2BD + MID : C_W2BD + 128] = m_w2
    wpack16[:MID, C_W3S] = m_w3[:, 0]
    wpack16[MID:, C_W3S + 1] = m_w3[:, 0]
    wpack16 = wpack16.astype(ml_dtypes.bfloat16)

    in_maps = []
    for c in range(NCORES):
        sl = slice(c * BPC, (c + 1) * BPC)
        in_maps.append(
            dict(
                wpack=wpack,
                wpack16=wpack16,
                xT=np.ascontiguousarray(xT[sl]),
                ab0=np.ascontiguousarray(ab0[sl]),
            )
        )
    return in_maps


def kernel(**inputs) -> np.ndarray:
    in_maps = _host_inputs(inputs)

    from concourse.bass_utils import run_bass_kernel_spmd

    nc = _build_bass()
    res = run_bass_kernel_spmd(
        nc, in_maps, core_ids=list(range(NCORES)),
        trace=bool(int(os.environ.get("KERNEL_TRACE", "0"))),
    )
    out = np.concatenate([r["out"] for r in res.results], axis=0)  # [B, N]
    if res.exec_time_ns is not None:
        print(f"HW exec time: {res.exec_time_ns} ns")
        print(f"mean exec time: {res.mean_exec_time_ns} ns")
    return out.astype(np.float32)


if __name__ == "__main__":
    nc = _build_bass()
    print("bass build OK")


# revision 21
# speedup vs baseline: 1.6353x; 1.0023x over previous
"""Trainium2 Bass kernel for GraphTransitionModel (GNN message passing).

Model (per batch element b, N=256 nodes):
  x[i]   = (obs[b,i], i/N)                              node features, 2-dim
  h1     = relu(A^T x_i + B^T x_j + a*w4 + b0)          messenger layer 1, 64
  h2     = relu(W1^T h1 + b1)                           64
  h3     = relu(W2^T h2 + b2)                           64
  m(i,j) = w3 . h3 + b3                                 scalar
  msg[i] = sum_j m(i,j)
  out    = MLP_updater([x_i, msg[i]])  (3->64->64->64->1)

Strategy: pure data parallel, 4 batch elements per core x 8 cores.

Device layout ("j-loop"): iterate over j; free dim carries all 256 i's.
Two j-rows (j and j+128) are stacked into 128 partitions; the 64x64
layers run as 128x128 block-diagonal bf16 matmuls (1 cycle/col on PE
vs 4 for fp32).  The sum over j is folded into PSUM accumulation of a
tiny third matmul (w3s stationary, start=False across the j-loop), so
no accumulator drains or free-dim reductions are needed.

Elementwise balance per 4-j chunk (free dim 1024):
  DVE: 4x h1 tensor_scalar (bf16 4x mode) + h3 cols [0:HS)
  ACT: h2 relu over [128,1024] + h3 cols [HS:1024)
mm3 reads h3 in 256-col pieces so each matmul sees a single-writer
tile.  The updater MLP runs once at the end over all 4 batches
(free dim 1024, layers 2-4 in bf16).

Sync-wait discipline (single sync-wait slot on matmul): constants via
packed DMAs absorbed by dummy PE matmuls; multi-writer tiles (Pd, Qs)
fenced through single DVE copies before the pair loop reads them.
"""

import os
import sys
import numpy as np

sys.path.insert(0, "/opt/trn_rl_repo")

B, N, MID = 32, 256, 64
NCORES = 8
BPC = B // NCORES  # batches per core = 4
HALF = N // 2  # 128 stacked j-iterations per batch
JCH = 4  # j's per chunk
NCH = HALF // JCH  # 32 chunks per batch
FREE = JCH * N  # 1024 free columns per chunk
HS = 512  # h3 column split: [0:HS) on DVE, [HS:FREE) on ACT

# fp32 wpack column layout
C_W0A = 0
C_W0B = 128
C_UW0 = 256
C_B1S = 320
C_B2S = 321
C_UB0 = 322
C_UB1 = 323
C_UB2 = 324
C_UB3 = 325
C_UW1 = 326
C_UW2 = 390
C_UW3 = 454
C_TOT32 = 455

# bf16 wpack16 column layout
C_W1BD = 0
C_W2BD = 128
C_W3S = 256
C_W0A16 = 258
C_W0B16 = 386
C_TOT16 = 514


def _build_bass():
    import concourse.bass as bass
    import concourse.bacc as bacc
    import concourse.tile as tile
    from concourse import mybir

    f32 = mybir.dt.float32
    bf16 = mybir.dt.bfloat16
    AF = mybir.ActivationFunctionType
    ALU = mybir.AluOpType

    nc = bacc.Bacc("TRN2", target_bir_lowering=False, num_devices=NCORES)

    wp_d = nc.declare_dram_parameter("wpack", [128, C_TOT32], f32, isOutput=False)
    wp16_d = nc.declare_dram_parameter("wpack16", [128, C_TOT16], bf16, isOutput=False)
    xT_d = nc.declare_dram_parameter("xT", [BPC, 2, N], f32, isOutput=False)
    ab0_d = nc.declare_dram_parameter("ab0", [BPC, MID, 1], f32, isOutput=False)
    out_d = nc.declare_dram_parameter("out", [BPC, N], f32, isOutput=True)

    with tile.TileContext(nc) as tc:
        with (
            tc.tile_pool(name="consts", bufs=1) as consts,
            tc.tile_pool(name="perb", bufs=2) as perb,
            tc.tile_pool(name="wk1", bufs=6) as wk1,
            tc.tile_pool(name="wk2", bufs=3) as wk2,
            tc.tile_pool(name="wk3a", bufs=3) as wk3a,
            tc.tile_pool(name="wk3b", bufs=3) as wk3b,
            tc.tile_pool(name="wkt", bufs=3) as wkt,
            tc.tile_pool(name="ps_z", bufs=3, space="PSUM") as ps_z,
            tc.tile_pool(name="ps_msg", bufs=1, space="PSUM") as ps_msg,
            tc.tile_pool(name="ps_small", bufs=1, space="PSUM") as ps_small,
        ):
            wp = consts.tile([128, C_TOT32], f32, tag="wpack")
            nc.sync.dma_start(out=wp[:], in_=wp_d[:])
            wp16 = consts.tile([128, C_TOT16], bf16, tag="wpack16")
            nc.sync.dma_start(out=wp16[:], in_=wp16_d[:])

            w0a = wp[0:2, C_W0A : C_W0A + 128]
            w0b = wp[0:2, C_W0B : C_W0B + 128]
            uw0 = wp[0:6, C_UW0 : C_UW0 + MID]
            b1s = wp[:, C_B1S : C_B1S + 1]
            b2s = wp[:, C_B2S : C_B2S + 1]
            ub0 = wp[0:MID, C_UB0 : C_UB0 + 1]
            ub1 = wp[0:MID, C_UB1 : C_UB1 + 1]
            ub2 = wp[0:MID, C_UB2 : C_UB2 + 1]
            ub3 = wp[0:1, C_UB3 : C_UB3 + 1]

            w1bd = wp16[:, C_W1BD : C_W1BD + 128]
            w2bd = wp16[:, C_W2BD : C_W2BD + 128]
            w3s = wp16[:, C_W3S : C_W3S + 2]
            w0a16 = wp16[0:2, C_W0A16 : C_W0A16 + 128]
            w0b16 = wp16[0:2, C_W0B16 : C_W0B16 + 128]
            uw1 = wp[0:MID, C_UW1 : C_UW1 + MID]
            uw2 = wp[0:MID, C_UW2 : C_UW2 + MID]
            uw3 = wp[0:MID, C_UW3 : C_UW3 + 1]

            # Dummy PE matmuls absorb the two const-DMA waits so later
            # matmuls (single sync-wait slot) only wait on their RAW
            # producer engine.
            psw = ps_small.tile([1, 1], f32, tag="pss")
            nc.tensor.matmul(psw[:], wp[0:1, 0:1], wp[0:1, 0:1], start=True, stop=True)
            psw2 = ps_small.tile([1, 1], f32, tag="pss")
            nc.tensor.matmul(
                psw2[:], wp16[0:1, 0:1], wp16[0:1, 0:1], start=True, stop=True
            )

            # updater input for all batches:
            # rows [obs, coor, msgA_even, msgB_even, msgA_odd, msgB_odd]
            uin = consts.tile([6, BPC * N], f32, tag="uin")

            def emit_setup(b):
                # w0a/w0b carry duplicated column blocks [A|A], [B|B], so the
                # first-layer matmuls emit both partition halves directly --
                # no partition-dup DMA and no Pd fence (single ACT writer).
                bcs = slice(b * N, (b + 1) * N)
                nc.sync.dma_start(out=uin[0:2, bcs], in_=xT_d[b])
                ab0s = perb.tile([128, 1], f32, tag="ab0s")
                src_ab = ab0_d[b]
                ab_bcast = bass.AP(
                    tensor=src_ab.tensor,
                    offset=src_ab.offset,
                    ap=[[0, 2]] + list(src_ab.ap),
                )
                nc.sync.dma_start(out=ab0s[:], in_=ab_bcast)

                uin16 = perb.tile([2, N], bf16, tag="uin16")
                nc.scalar.copy(uin16[:], uin[0:2, bcs])
                psP = ps_small.tile([128, N], f32, tag="pss")
                nc.tensor.matmul(psP[:], w0a16, uin16[:], start=True, stop=True)
                Pd = perb.tile([128, N], bf16, tag="Pd")
                nc.scalar.copy(Pd[:], psP[:])

                psQ = ps_small.tile([128, N], f32, tag="pss")
                nc.tensor.matmul(psQ[:], w0b16, uin16[:], start=True, stop=True)
                qtmp = perb.tile([128, N], f32, tag="qtmp")
                nc.scalar.activation(qtmp[:], psQ[:], AF.Identity, bias=ab0s)
                Qs = perb.tile([128, HALF], f32, tag="Qs")
                nc.sync.dma_start(out=Qs[0:MID, :], in_=qtmp[0:MID, 0:HALF])
                nc.sync.dma_start(out=Qs[MID:128, :], in_=qtmp[MID:128, HALF:N])

                # DVE fence: Qs has two DMA writers
                Qs2 = perb.tile([128, HALF], f32, tag="Qs2")
                nc.vector.tensor_copy(Qs2[:], Qs[:])

                # msg accumulator: partitions = j-halves, cols = even/odd j
                msg_ps = ps_msg.tile([2, 2 * N], f32, tag="msg")
                return (Pd, Qs2, msg_ps)

            # ---- flat 4-stage pipeline over all BPC*NCH chunks ----
            # chunk x: quad(x) @ body x [DVE] -> mm1(x) @ x+1 [PE] ->
            # h2(x) @ x+1 [ACT] -> mm2(x) @ x+2 [PE] -> h3(x) @ x+3
            # [DVE+ACT] -> mm3(x) @ x+3 [PE].  Every consumer runs a full
            # body after its producer, so the period is set by the busiest
            # engine, not by cross-engine dependency round trips.
            T = BPC * NCH
            ctx = {0: emit_setup(0)}
            h1q, z2q, h2q, z3q, h3q = {}, {}, {}, {}, {}
            for t in range(T + 4):
                # stage 4a: h3(t-3) relu split DVE/ACT
                if 3 <= t < T + 3:
                    x = t - 3
                    z3p = z3q.pop(x)
                    h3a = wk3a.tile([128, HS], bf16, tag="h3a")
                    nc.vector.tensor_scalar(
                        h3a[:], z3p[:, 0:HS], b2s, 0.0, ALU.add, ALU.max
                    )
                    h3b = wk3b.tile([128, FREE - HS], bf16, tag="h3b")
                    nc.scalar.activation(
                        h3b[:], z3p[:, HS:FREE], AF.Relu, bias=b2s
                    )
                    h3q[x] = (h3a, h3b)
                # stage 1: h1 quad(t) on DVE
                if t < T:
                    bq, cq = divmod(t, NCH)
                    Pd2, Qs2, _ = ctx[bq]
                    jb = cq * JCH
                    h1a = wk1.tile([128, 2 * N], bf16, tag="h1a")
                    nc.vector.tensor_scalar(
                        h1a[:, 0:N], Pd2[:], Qs2[:, jb : jb + 1],
                        0.0, ALU.add, ALU.max,
                    )
                    nc.vector.tensor_scalar(
                        h1a[:, N : 2 * N], Pd2[:], Qs2[:, jb + 1 : jb + 2],
                        0.0, ALU.add, ALU.max,
                    )
                    h1b = wk1.tile([128, 2 * N], bf16, tag="h1b")
                    nc.vector.tensor_scalar(
                        h1b[:, 0:N], Pd2[:], Qs2[:, jb + 2 : jb + 3],
                        0.0, ALU.add, ALU.max,
                    )
                    nc.vector.tensor_scalar(
                        h1b[:, N : 2 * N], Pd2[:], Qs2[:, jb + 3 : jb + 4],
                        0.0, ALU.add, ALU.max,
                    )
                    h1q[t] = (h1a, h1b)
                # stage 2a: mm1(t-1) on PE (z2 single-buffered: its reader
                # h2(t-1) finished a body before mm1(t) rewrites it)
                if 1 <= t <= T:
                    x = t - 1
                    p1a, p1b = h1q.pop(x)
                    z2 = ps_z.tile([128, FREE], f32, tag="z")
                    nc.tensor.matmul(
                        z2[:, 0 : 2 * N], w1bd, p1a[:], start=True, stop=True
                    )
                    nc.tensor.matmul(
                        z2[:, 2 * N : FREE], w1bd, p1b[:], start=True, stop=True
                    )
                    z2q[x] = z2
                # stage 3: mm2(t-2) on PE
                if 2 <= t <= T + 1:
                    x = t - 2
                    h2c = h2q.pop(x)
                    z3 = ps_z.tile([128, FREE], f32, tag="z")
                    nc.tensor.matmul(
                        z3[:, 0 : 2 * N], w2bd, h2c[:, 0 : 2 * N],
                        start=True, stop=True,
                    )
                    nc.tensor.matmul(
                        z3[:, 2 * N : FREE], w2bd, h2c[:, 2 * N : FREE],
                        start=True, stop=True,
                    )
                    z3q[x] = z3
                # stage 4b: mm3(t-3) accumulate + batch drain
                if 3 <= t < T + 3:
                    x = t - 3
                    bx, cx = divmod(x, NCH)
                    p3a, p3b = h3q.pop(x)
                    msg_x = ctx[bx][2]
                    nc.tensor.matmul(
                        msg_x[:], w3s, p3a[:],
                        start=(cx == 0), stop=False, skip_group_check=True,
                    )
                    nc.tensor.matmul(
                        msg_x[:], w3s, p3b[:],
                        start=False, stop=(cx == NCH - 1),
                        skip_group_check=True,
                    )
                    if cx == NCH - 1:
                        # msg PSUM -> SBUF -> uin rows 2:6 (DMA crosses
                        # partitions)
                        bcx = slice(bx * N, (bx + 1) * N)
                        msg_sb = perb.tile([2, 2 * N], f32, tag="msg_sb")
                        nc.scalar.copy(msg_sb[:], msg_x[:])
                        nc.sync.dma_start(out=uin[2:4, bcx], in_=msg_sb[:, 0:N])
                        nc.sync.dma_start(
                            out=uin[4:6, bcx], in_=msg_sb[:, N : 2 * N]
                        )
                        del ctx[bx]
                # stage 2b: h2(t-1) on ACT
                if 1 <= t <= T:
                    x = t - 1
                    z2c = z2q.pop(x)
                    h2 = wk2.tile([128, FREE], bf16, tag="h2")
                    nc.scalar.activation(h2[:], z2c[:], AF.Relu, bias=b1s)
                    h2q[x] = h2
                # prefetch next batch's setup two bodies early
                bn, rn = divmod(t + 2, NCH)
                if rn == 0 and 1 <= bn < BPC:
                    ctx[bn] = emit_setup(bn)

            # ---- updater MLP, all batches at once (free = BPC*N) ----
            # fp32 throughout (bf16 activations here cost 2.8e-2 rel err);
            # two column-halves pipeline the PE matmuls against ACT relus.
            FB = BPC * N
            qs4 = [slice(k * N, (k + 1) * N) for k in range(BPC)]
            psu1 = ps_z.tile([MID, FB], f32, tag="z")
            for s in qs4:
                nc.tensor.matmul(psu1[:, s], uw0, uin[:, s], start=True, stop=True)
            t1 = wkt.tile([MID, FB], f32, tag="t1")
            psu2 = ps_z.tile([MID, FB], f32, tag="z")
            t2 = wkt.tile([MID, FB], f32, tag="t2")
            psu3 = ps_z.tile([MID, FB], f32, tag="z")
            t3 = wkt.tile([MID, FB], f32, tag="t3")
            pso = ps_z.tile([1, FB], f32, tag="z")
            orow = consts.tile([1, FB], f32, tag="orow")
            for s in qs4:
                nc.scalar.activation(t1[:, s], psu1[:, s], AF.Relu, bias=ub0)
                nc.tensor.matmul(psu2[:, s], uw1, t1[:, s], start=True, stop=True)
            for s in qs4:
                nc.scalar.activation(t2[:, s], psu2[:, s], AF.Relu, bias=ub1)
                nc.tensor.matmul(psu3[:, s], uw2, t2[:, s], start=True, stop=True)
            for s in qs4:
                nc.scalar.activation(t3[:, s], psu3[:, s], AF.Relu, bias=ub2)
                nc.tensor.matmul(pso[:, s], uw3, t3[:, s], start=True, stop=True)
            for s in qs4:
                nc.scalar.activation(orow[:, s], pso[:, s], AF.Identity, bias=ub3)
            nc.sync.dma_start(out=out_d[:, :], in_=orow[:])

    nc.compile()
    return nc


def _host_inputs(inputs):
    import ml_dtypes

    g = lambda k: np.asarray(inputs[k], np.float32)
    obs, action = g("obs"), g("action")
    m_w0, m_b0, m_w1, m_b1 = g("m_w0"), g("m_b0"), g("m_w1"), g("m_b1")
    m_w2, m_b2, m_w3, m_b3 = g("m_w2"), g("m_b2"), g("m_w3"), g("m_b3")
    u_w0, u_b0, u_w1, u_b1 = g("u_w0"), g("u_b0"), g("u_w1"), g("u_b1")
    u_w2, u_b2, u_w3, u_b3 = g("u_w2"), g("u_b2"), g("u_w3"), g("u_b3")

    coor = np.arange(N, dtype=np.float32) / N
    xT = np.stack([obs, np.broadcast_to(coor, obs.shape)], axis=1)  # [B, 2, N]
    # per-batch action/bias for Q: a*w4 + b0
    ab0 = (action[:, None] * m_w0[4] + m_b0).astype(np.float32)[..., None]

    wpack = np.zeros((128, C_TOT32), np.float32)
    wpack[0:2, C_W0A : C_W0A + MID] = m_w0[0:2]
    wpack[0:2, C_W0A + MID : C_W0A + 128] = m_w0[0:2]
    wpack[0:2, C_W0B : C_W0B + MID] = m_w0[2:4]
    wpack[0:2, C_W0B + MID : C_W0B + 128] = m_w0[2:4]
    wpack[0:2, C_UW0 : C_UW0 + MID] = u_w0[0:2]
    for _r in (2, 3, 4, 5):
        wpack[_r, C_UW0 : C_UW0 + MID] = u_w0[2]
    wpack[:MID, C_B1S] = m_b1
    wpack[MID:, C_B1S] = m_b1
    wpack[:MID, C_B2S] = m_b2
    wpack[MID:, C_B2S] = m_b2
    # msg rows exclude the +N*b3 term; fold it into the updater bias
    wpack[:MID, C_UB0] = u_b0 + N * float(m_b3[0]) * u_w0[2]
    wpack[:MID, C_UB1] = u_b1
    wpack[:MID, C_UB2] = u_b2
    wpack[0, C_UB3] = float(u_b3[0])
    wpack[:MID, C_UW1 : C_UW1 + MID] = u_w1
    wpack[:MID, C_UW2 : C_UW2 + MID] = u_w2
    wpack[:MID, C_UW3] = u_w3[:, 0]

    wpack16 = np.zeros((128, C_TOT16), np.float32)
    wpack16[:MID, C_W1BD : C_W1BD + MID] = m_w1
    wpack16[MID:, C_W1BD + MID : C_W1BD + 128] = m_w1
    wpack16[:MID, C_W2BD : C_W2BD + MID] = m_w2
    wpack16[MID:, C_W2BD + MID : C_W2BD + 128] = m_w2
    wpack16[:MID, C_W3S] = m_w3[:, 0]
    wpack16[MID:, C_W3S + 1] = m_w3[:, 0]
    wpack16[0:2, C_W0A16 : C_W0A16 + MID] = m_w0[0:2]
    wpack16[0:2, C_W0A16 + MID : C_W0A16 + 128] = m_w0[0:2]
    wpack16[0:2, C_W0B16 : C_W0B16 + MID] = m_w0[2:4]
    wpack16[0:2, C_W0B16 + MID : C_W0B16 + 128] = m_w0[2:4]
    wpack16 = wpack16.astype(ml_dtypes.bfloat16)

    in_maps = []
    for c in range(NCORES):
        sl = slice(c * BPC, (c + 1) * BPC)
        in_maps.append(
            dict(
                wpack=wpack,
                wpack16=wpack16,
                xT=np.ascontiguousarray(xT[sl]),
                ab0=np.ascontiguousarray(ab0[sl]),
            )
        )
    return in_maps


def kernel(**inputs) -> np.ndarray:
    in_maps = _host_inputs(inputs)

    from concourse.bass_utils import run_bass_kernel_spmd

    nc = _build_bass()
    res = run_bass_kernel_spmd(
        nc, in_maps, core_ids=list(range(NCORES)),
        trace=bool(int(os.environ.get("KERNEL_TRACE", "0"))),
    )
    out = np.concatenate([r["out"] for r in res.results], axis=0)  # [B, N]
    if res.exec_time_ns is not None:
        print(f"HW exec time: {res.exec_time_ns} ns")
        print(f"mean exec time: {res.mean_exec_time_ns} ns")
    return out.astype(np.float32)


if __name__ == "__main__":
    nc = _build_bass()
    print("bass build OK")
